# revision 1
# baseline (speedup 1.0000x reference)
"""MLA attention (DeepSeek-style) Trainium2 Bass kernel, 8-core SPMD.

Sharding: core c handles batch b = c//4 and head-group g = c%4 (4 of 16 heads).
Down-projections are replicated per batch; up-projections / attention / o-proj
are head-parallel. Host sums the 4 partial o-projections per batch.

Device dataflow (per core, transposed-activation layout, S processed in 4
chunks of 512):
  xT (host-transposed, bf16) -> q_latT/kv_latT (bf16 matmuls -> f32r latents)
  -> per-head qT/kT [128=HD, S] bf16 tiles assembled from PSUM (nope rows
  0:64, rope rows 64:128; q uses host-concatenated Wq_up|Wq_rope columns),
  RoPE via host-baked cos/sin tables -> causal flash attention per head:
  scoresT [j,i] matmuls, exp on ScalarE (scale fused), unnormalized attnout +
  ones-matmul row sums, normalize by broadcast reciprocal -> o-projection ->
  partial [S, D] f32 out.
"""

import numpy as np
import ml_dtypes

import concourse.bacc as bacc
import concourse.mybir as mybir
import concourse.tile as tile
from concourse.bass_utils import run_bass_kernel_spmd

F32 = mybir.dt.float32
F32R = mybir.dt.float32r
BF16 = mybir.dt.bfloat16

B, S, D = 2, 2048, 2048
H, HD = 16, 128
RD, ND = 64, 64
KVR, QR = 512, 1024
BASE = 10000.0
HLOC = 4                 # heads per core
CHUNK = 512
NCHUNK = S // CHUNK      # 4
P = 128
SCALE = HD ** -0.5

_BF16 = ml_dtypes.bfloat16


def _build():
    nc = bacc.Bacc("TRN2", target_bir_lowering=False, debug=False)

    xT = nc.dram_tensor("xT", [D, S], BF16, kind="ExternalInput").ap()
    wqd = nc.dram_tensor("wqd", [D, QR], BF16, kind="ExternalInput").ap()
    wkvd = nc.dram_tensor("wkvd", [D, KVR], BF16, kind="ExternalInput").ap()
    wkr = nc.dram_tensor("wkr", [D, HLOC * RD], BF16, kind="ExternalInput").ap()
    wqcat = nc.dram_tensor("wqcat", [QR, HLOC * HD], F32, kind="ExternalInput").ap()
    wkup = nc.dram_tensor("wkup", [KVR, HLOC * ND], F32, kind="ExternalInput").ap()
    wvup = nc.dram_tensor("wvup", [KVR, HLOC * HD], F32, kind="ExternalInput").ap()
    wo = nc.dram_tensor("wo", [HLOC * HD, D], BF16, kind="ExternalInput").ap()
    cosr = nc.dram_tensor("cosr", [P, S], F32, kind="ExternalInput").ap()
    sinr = nc.dram_tensor("sinr", [P, S], F32, kind="ExternalInput").ap()
    maskd = nc.dram_tensor("maskd", [P, 4 * CHUNK], BF16, kind="ExternalInput").ap()
    o_part = nc.dram_tensor("o_part", [S, D], F32, kind="ExternalOutput").ap()

    xT_r = xT.rearrange("(dt p) s -> p dt s", p=P)          # [128, 16, S]
    wqd_r = wqd.rearrange("(dt p) q -> p dt q", p=P)        # [128, 16, 1024]
    wkvd_r = wkvd.rearrange("(dt p) q -> p dt q", p=P)      # [128, 16, 512]
    wkr_r = wkr.rearrange("(dt p) q -> p dt q", p=P)        # [128, 16, 256]
    wqcat_r = wqcat.rearrange("(qt p) c -> p qt c", p=P)    # [128, 8, 512]
    wkup_r = wkup.rearrange("(kt p) c -> p kt c", p=P)      # [128, 4, 256]
    wvup_r = wvup.rearrange("(kt p) c -> p kt c", p=P)      # [128, 4, 512]
    wo_r = wo.rearrange("(kt p) d -> p kt d", p=P)          # [128, 4, 2048]
    mask_r = maskd.rearrange("p (r i) -> p r i", r=4)       # [128, 4, 512]
    o_r = o_part.rearrange("(st p) d -> p st d", p=P)       # [128, 16, 2048]

    with tile.TileContext(nc) as tc:
        with (
            tc.tile_pool(name="persist", bufs=1) as pp,
            tc.tile_pool(name="acts", bufs=1) as ap_,
            tc.tile_pool(name="wstream", bufs=3) as wp,
            tc.tile_pool(name="wbig", bufs=1) as wb,
            tc.tile_pool(name="tabs", bufs=1) as tp,
            tc.tile_pool(name="rope", bufs=2) as rp,
            tc.tile_pool(name="attn", bufs=2) as atp,
            tc.tile_pool(name="outp", bufs=2) as op_,
            tc.tile_pool(name="aoutp", bufs=2) as aop,
            tc.tile_pool(name="psA", bufs=2, space="PSUM") as psA,
            tc.tile_pool(name="psS", bufs=2, space="PSUM") as psS,
            tc.tile_pool(name="psD", bufs=2, space="PSUM") as psD,
            tc.tile_pool(name="psO", bufs=2, space="PSUM") as psO,
        ):
            # ---------------- persistent tiles ----------------
            kT = pp.tile([P, HLOC, S], BF16, tag="kT")            # per-head K^T
            vnat = pp.tile([P, S // P, HLOC * HD], BF16, tag="vnat")  # V natural
            masks = pp.tile([P, 4, CHUNK], BF16, tag="masks")
            ones = pp.tile([P, P], BF16, tag="ones")
            wo_t = pp.tile([P, 4, D], BF16, tag="wo")             # resident Wo

            nc.vector.memset(ones[:], 1.0)

            def o_proj(ic, aout):
                for st in range(CHUNK // P):
                    for dc in range(D // CHUNK):
                        ps = psA.tile([P, CHUNK], F32, tag="psA")
                        for kt in range(HLOC):
                            nc.tensor.matmul(
                                ps[:], aout[:, kt, P * st:P * (st + 1)],
                                wo_t[:, kt, CHUNK * dc:CHUNK * (dc + 1)],
                                start=(kt == 0), stop=(kt == HLOC - 1))
                        osb = op_.tile([P, CHUNK], F32, tag="osb")
                        nc.scalar.copy(osb[:], ps[:])
                        nc.scalar.dma_start(
                            o_r[:, ic * (CHUNK // P) + st,
                                CHUNK * dc:CHUNK * (dc + 1)], osb[:])

            def rope_store(ps_pe, b, dst_pe, cos_c, sin_c):
                """ps_pe: [64, CHUNK] psum AP at partition base b in {0, 64}
                (pre-rope pe rows of one head). 4 DVE ops, windows chosen so
                the sign-baked sin/cos tables align with tmp/scr rows; only
                psum reads / the final store cross partition quadrants.
                dst_pe = bf16 tile rows [64:128]."""
                tmp = rp.tile([P, CHUNK], F32, tag="ropetmp")
                scr = rp.tile([P, CHUNK], F32, tag="ropescr")
                nc.vector.tensor_tensor(tmp[b:b + 32, :], ps_pe[32:64, :],
                                        sin_c[b:b + 32, :], mybir.AluOpType.mult)
                nc.vector.tensor_tensor(tmp[b + 32:b + 64, :], ps_pe[0:32, :],
                                        sin_c[b + 32:b + 64, :],
                                        mybir.AluOpType.mult)
                nc.vector.tensor_tensor(scr[b:b + 64, :], ps_pe[:],
                                        cos_c[b:b + 64, :], mybir.AluOpType.mult)
                nc.vector.tensor_tensor(dst_pe, scr[b:b + 64, :],
                                        tmp[b:b + 64, :], mybir.AluOpType.add)

            # ---------------- chunk loop ----------------
            for ic in range(NCHUNK):
                sl = slice(ic * CHUNK, (ic + 1) * CHUNK)

                xc = ap_.tile([P, D // P, CHUNK], BF16, tag="xc")
                for dt_ in range(D // P):
                    nc.sync.dma_start(xc[:, dt_, :], xT_r[:, dt_, sl])

                cos_c = tp.tile([P, CHUNK], F32, tag="cos")
                sin_c = tp.tile([P, CHUNK], F32, tag="sin")
                nc.scalar.dma_start(cos_c[:], cosr[:, sl])
                nc.scalar.dma_start(sin_c[:], sinr[:, sl])
                if ic == 0:
                    nc.scalar.dma_start(masks[:], mask_r[:])

                # ---- q_latT [1024, CHUNK] (f32r) ----
                qlat = ap_.tile([P, QR // P, CHUNK], F32R, tag="qlat")
                for cp in range(QR // P // 2):          # c-tile pairs
                    ws = wp.tile([P, D // P, 2 * P], BF16, tag="wstrip")
                    nc.sync.dma_start(
                        ws[:], wqd_r[:, :, 2 * P * cp:2 * P * (cp + 1)])
                    for ci in range(2):
                        c = 2 * cp + ci
                        ps = psA.tile([P, CHUNK], F32, tag="psA")
                        for dt_ in range(D // P):
                            nc.tensor.matmul(
                                ps[:], ws[:, dt_, P * ci:P * (ci + 1)],
                                xc[:, dt_, :],
                                start=(dt_ == 0), stop=(dt_ == D // P - 1))
                        nc.scalar.copy(qlat[:, c, :], ps[:])

                # ---- kv_latT [512, CHUNK] (f32r) ----
                kvlat = ap_.tile([P, KVR // P, CHUNK], F32R, tag="kvlat")
                for cp in range(KVR // P // 2):
                    ws = wp.tile([P, D // P, 2 * P], BF16, tag="wstrip")
                    nc.sync.dma_start(
                        ws[:], wkvd_r[:, :, 2 * P * cp:2 * P * (cp + 1)])
                    for ci in range(2):
                        c = 2 * cp + ci
                        ps = psA.tile([P, CHUNK], F32, tag="psA")
                        for dt_ in range(D // P):
                            nc.tensor.matmul(
                                ps[:], ws[:, dt_, P * ci:P * (ci + 1)],
                                xc[:, dt_, :],
                                start=(dt_ == 0), stop=(dt_ == D // P - 1))
                        nc.scalar.copy(kvlat[:, c, :], ps[:])

                # ---- k_pe: 2 c-tiles of 128 = (heads 2a, 2a+1) rope dims ----
                ws_kr = wp.tile([P, D // P, 2 * P], BF16, tag="wstrip")
                nc.sync.dma_start(ws_kr[:], wkr_r[:])
                for a in range(2):
                    ps = psA.tile([P, CHUNK], F32, tag="psA")
                    for dt_ in range(D // P):
                        nc.tensor.matmul(
                            ps[:], ws_kr[:, dt_, P * a:P * (a + 1)],
                            xc[:, dt_, :],
                            start=(dt_ == 0), stop=(dt_ == D // P - 1))
                    rope_store(ps[0:64, :], 0, kT[64:128, 2 * a, sl], cos_c, sin_c)
                    rope_store(ps[64:128, :], 64, kT[64:128, 2 * a + 1, sl],
                               cos_c, sin_c)

                # ---- k_nope: 2 c-tiles = (heads 2a, 2a+1) nope dims ----
                ws_kn = wp.tile([P, KVR // P, 2 * P], F32R, tag="wstrip")
                nc.sync.dma_start(ws_kn[:], wkup_r[:].bitcast(F32R))
                for a in range(2):
                    ps = psA.tile([P, CHUNK], F32, tag="psA")
                    for kt in range(KVR // P):
                        nc.tensor.matmul(
                            ps[:], ws_kn[:, kt, P * a:P * (a + 1)],
                            kvlat[:, kt, :],
                            start=(kt == 0), stop=(kt == KVR // P - 1))
                    nc.vector.tensor_copy(kT[0:64, 2 * a, sl], ps[0:64, :])
                    nc.vector.tensor_copy(kT[0:64, 2 * a + 1, sl], ps[64:128, :])

                # ---- q heads: c-tile h = head h [nope64 | pe64] ----
                qTi = ap_.tile([P, HLOC, CHUNK], BF16, tag="qTi")
                for hp in range(HLOC // 2):
                    ws = wp.tile([P, QR // P, 2 * P], F32R, tag="wstrip")
                    nc.sync.dma_start(
                        ws[:],
                        wqcat_r[:, :, 2 * P * hp:2 * P * (hp + 1)].bitcast(F32R))
                    for ci in range(2):
                        h = 2 * hp + ci
                        ps = psA.tile([P, CHUNK], F32, tag="psA")
                        for qt in range(QR // P):
                            nc.tensor.matmul(
                                ps[:], ws[:, qt, P * ci:P * (ci + 1)],
                                qlat[:, qt, :],
                                start=(qt == 0), stop=(qt == QR // P - 1))
                        nc.vector.tensor_copy(qTi[0:64, h, :], ps[0:64, :])
                        rope_store(ps[64:128, :], 64, qTi[64:128, h, :], cos_c, sin_c)

                if ic == 0:
                    # resident Wo load, deferred so it doesn't crowd the
                    # critical first-chunk x/weight DMAs
                    for kt in range(4):
                        nc.sync.dma_start(wo_t[:, kt, :], wo_r[:, kt, :])

                # ---- v natural [CHUNK, 512] ----
                ws_v = wb.tile([P, KVR // P, HLOC * HD], F32R, tag="wvup")
                nc.sync.dma_start(ws_v[:], wvup_r[:].bitcast(F32R))
                for st in range(CHUNK // P):
                    ps = psA.tile([P, HLOC * HD], F32, tag="psA")
                    for kt in range(KVR // P):
                        nc.tensor.matmul(
                            ps[:], kvlat[:, kt, P * st:P * (st + 1)],
                            ws_v[:, kt, :],
                            start=(kt == 0), stop=(kt == KVR // P - 1))
                    nc.vector.tensor_copy(vnat[:, ic * (CHUNK // P) + st, :], ps[:])

                # ---- o-projection of the PREVIOUS chunk: PE work to cover
                # the DVE rope/normalize backlog of this chunk's projections
                if ic > 0:
                    o_proj(ic - 1, prev_aout)

                # ---- attention for this query chunk ----
                aout = aop.tile([P, HLOC, CHUNK], BF16, tag="aout")
                jt_max = (ic + 1) * (CHUNK // P)
                for h in range(HLOC):
                    psd = psD.tile([P, CHUNK], F32, tag="psD")
                    pso = psO.tile([P, CHUNK], F32, tag="psO")
                    for jt in range(jt_max):
                        pss = psS.tile([P, CHUNK], F32, tag="psS")
                        nc.tensor.matmul(
                            pss[:], kT[:, h, P * jt:P * (jt + 1)], qTi[:, h, :],
                            start=True, stop=True)
                        at = atp.tile([P, CHUNK], BF16, tag="attnT")
                        nc.scalar.activation(
                            at[:], pss[:], mybir.ActivationFunctionType.Exp,
                            scale=SCALE)
                        r = jt - ic * (CHUNK // P)
                        if r >= 0:  # diagonal tile: causal mask
                            nc.vector.tensor_tensor(
                                at[:], at[:], masks[:, r, :],
                                mybir.AluOpType.mult)
                        nc.tensor.matmul(psd[:], ones[:], at[:],
                                         start=(jt == 0), stop=(jt == jt_max - 1))
                        nc.tensor.matmul(
                            pso[:], vnat[:, jt, HD * h:HD * (h + 1)], at[:],
                            start=(jt == 0), stop=(jt == jt_max - 1))
                    rec = atp.tile([P, CHUNK], F32, tag="recip")
                    nc.vector.reciprocal_approx_fast(rec[:], psd[:])
                    nc.vector.tensor_tensor(aout[:, h, :], pso[:], rec[:],
                                            mybir.AluOpType.mult)
                prev_aout = aout

            o_proj(NCHUNK - 1, prev_aout)
    nc.compile()
    return nc


_NC = None


def _get_nc():
    global _NC
    if _NC is None:
        _NC = _build()
    return _NC


def _host_prep(x, Wq_down, Wq_up, Wq_rope, Wkv_down, Wk_up, Wk_rope, Wv_up, Wo):
    """Build the 8 per-core input maps (all host-side layout prep)."""
    # rope tables, replicated to 128 partitions with NeoX sign baked into sin
    half = RD // 2
    inv_freq = 1.0 / (BASE ** (np.arange(half, dtype=np.float64) / half))
    ang = np.arange(S, dtype=np.float64)[None, :] * inv_freq[:, None]  # [32, S]
    cos32 = np.cos(ang)
    sin32 = np.sin(ang)
    cosr = np.tile(cos32, (4, 1)).astype(np.float32)                   # [128,S]
    sinr = np.concatenate([-sin32, sin32, -sin32, sin32], 0).astype(np.float32)

    # causal masks for diagonal (j-tile, i-chunk) pairs: r = 0..3
    pidx = np.arange(P)[:, None]
    iidx = np.arange(CHUNK)[None, :]
    masks = np.stack([(pidx + P * r <= iidx) for r in range(4)], axis=1)
    maskd = masks.astype(_BF16).reshape(P, 4 * CHUNK)

    xT = [np.ascontiguousarray(x[b].T).astype(_BF16) for b in range(B)]
    wqd = Wq_down.astype(_BF16)
    wkvd = Wkv_down.astype(_BF16)

    in_maps = []
    for c in range(8):
        b, g = divmod(c, 4)
        heads = range(HLOC * g, HLOC * (g + 1))
        wqcat = np.empty((QR, HLOC * HD), np.float32)
        for i, h in enumerate(heads):
            wqcat[:, i * HD:i * HD + ND] = Wq_up[:, h * ND:(h + 1) * ND]
            wqcat[:, i * HD + ND:(i + 1) * HD] = Wq_rope[:, h * RD:(h + 1) * RD]
        in_maps.append({
            "xT": xT[b],
            "wqd": wqd,
            "wkvd": wkvd,
            "wkr": np.ascontiguousarray(
                Wk_rope[:, g * HLOC * RD:(g + 1) * HLOC * RD]).astype(_BF16),
            "wqcat": wqcat,
            "wkup": np.ascontiguousarray(
                Wk_up[:, g * HLOC * ND:(g + 1) * HLOC * ND], dtype=np.float32),
            "wvup": np.ascontiguousarray(
                Wv_up[:, g * HLOC * HD:(g + 1) * HLOC * HD], dtype=np.float32),
            "wo": np.ascontiguousarray(
                Wo[g * HLOC * HD:(g + 1) * HLOC * HD, :]).astype(_BF16),
            "cosr": cosr,
            "sinr": sinr,
            "maskd": maskd,
        })
    return in_maps


def kernel(x, Wq_down, Wq_up, Wq_rope, Wkv_down, Wk_up, Wk_rope, Wv_up, Wo,
           _trace=False, _trace_kwargs=None):
    x = np.asarray(x, dtype=np.float32)
    args = [np.asarray(a, dtype=np.float32) for a in
            (Wq_down, Wq_up, Wq_rope, Wkv_down, Wk_up, Wk_rope, Wv_up, Wo)]
    in_maps = _host_prep(x, *args)
    nc = _get_nc()
    res = run_bass_kernel_spmd(nc, in_maps, core_ids=list(range(8)),
                               trace=_trace, **(_trace_kwargs or {}))
    kernel._last_results = res
    out = np.zeros((B, S, D), np.float32)
    for c in range(8):
        out[c // 4] += res.results[c]["o_part"]
    return out



# revision 2
# speedup vs baseline: 1.3351x; 1.3351x over previous
"""MLA attention (DeepSeek-style) Trainium2 Bass kernel, 8-core SPMD.

Sharding: core c handles batch b = c//4 and head-group g = c%4 (4 of 16 heads).
All low-rank projections are fused on the host (Wq_down@Wq_up etc.), so every
core runs a single head-parallel projection x @ Wqk [D, 1024] (per-head
[q_nope|q_pe|k_nope|k_pe] columns) + x @ Wv [D, 512] with ZERO replicated
work, then causal flash attention for its 4 heads and a partial o-projection.
Host sums the 4 partial o-projections per batch.

Device dataflow (per core, transposed-activation layout, S processed in 4
chunks of 512):
  xT (host-tiled, bf16) -> per-head qT/kT [128=HD, S] bf16 tiles straight from
  PSUM (nope rows 0:64, rope rows 64:128), RoPE via host-baked cos/sin tables
  -> V in natural layout via swapped-operand matmuls -> causal flash attention
  per head: scoresT [j,i] matmuls, exp on ScalarE (scale fused), unnormalized
  attnout + ones-matmul row sums, normalize by broadcast reciprocal ->
  o-projection -> partial [S, D] f32 out.

All DRAM tensors are host-pre-tiled to [128, ...] partition-major layout so
every DMA moves >=4KB contiguous per partition (fat descriptors).
"""

import numpy as np
import ml_dtypes

import concourse.bacc as bacc
import concourse.mybir as mybir
import concourse.tile as tile
from concourse.bass_utils import run_bass_kernel_spmd

F32 = mybir.dt.float32
BF16 = mybir.dt.bfloat16

B, S, D = 2, 2048, 2048
H, HD = 16, 128
RD, ND = 64, 64
KVR, QR = 512, 1024
BASE = 10000.0
HLOC = 4                 # heads per core
CHUNK = 512
NCHUNK = S // CHUNK      # 4
P = 128
DT = D // P              # 16 contraction tiles
NCT = 2 * HLOC           # 8 projection c-tiles (4 q heads + 4 k heads)
SCALE = HD ** -0.5

_BF16 = ml_dtypes.bfloat16


def _build():
    nc = bacc.Bacc("TRN2", target_bir_lowering=False, debug=False)

    # pre-tiled layouts: free index order is given in the rearrange below
    xt = nc.dram_tensor("xt", [P, NCHUNK * DT * CHUNK], BF16,
                        kind="ExternalInput").ap()
    wqk = nc.dram_tensor("wqk", [P, NCT * DT * P], BF16,
                         kind="ExternalInput").ap()
    wv = nc.dram_tensor("wv", [P, DT * HLOC * HD], BF16,
                        kind="ExternalInput").ap()
    wo = nc.dram_tensor("wo", [P, HLOC * D], BF16, kind="ExternalInput").ap()
    cosr = nc.dram_tensor("cosr", [P, S], F32, kind="ExternalInput").ap()
    sinr = nc.dram_tensor("sinr", [P, S], F32, kind="ExternalInput").ap()
    maskd = nc.dram_tensor("maskd", [P, 4 * CHUNK], BF16,
                           kind="ExternalInput").ap()
    o_part = nc.dram_tensor("o_part", [S, D], F32, kind="ExternalOutput").ap()

    xt_r = xt.rearrange("p (ic dt s) -> p ic dt s", ic=NCHUNK, dt=DT)
    wqk_r = wqk.rearrange("p (ct dt c) -> p ct dt c", ct=NCT, dt=DT)
    wv_r = wv.rearrange("p (dt c) -> p dt c", dt=DT)
    wo_r = wo.rearrange("p (kt d) -> p kt d", kt=HLOC)
    mask_r = maskd.rearrange("p (r i) -> p r i", r=4)
    o_r = o_part.rearrange("(st p) d -> p st d", p=P)       # [128, 16, 2048]

    with tile.TileContext(nc) as tc:
        with (
            tc.tile_pool(name="persist", bufs=1) as pp,
            tc.tile_pool(name="acts", bufs=2) as ap_,
            tc.tile_pool(name="rope", bufs=2) as rp,
            tc.tile_pool(name="attn", bufs=2) as atp,
            tc.tile_pool(name="outp", bufs=2) as op_,
            tc.tile_pool(name="aoutp", bufs=2) as aop,
            tc.tile_pool(name="psA", bufs=2, space="PSUM") as psA,
            tc.tile_pool(name="psS", bufs=2, space="PSUM") as psS,
            tc.tile_pool(name="psD", bufs=2, space="PSUM") as psD,
            tc.tile_pool(name="psO", bufs=2, space="PSUM") as psO,
        ):
            # ---------------- persistent tiles ----------------
            kT = pp.tile([P, HLOC, S], BF16, tag="kT")            # per-head K^T
            vnat = pp.tile([P, S // P, HLOC * HD], BF16, tag="vnat")  # V natural
            masks = pp.tile([P, 4, CHUNK], BF16, tag="masks")
            ones = pp.tile([P, P], BF16, tag="ones")
            wqkt = pp.tile([P, NCT, DT, P], BF16, tag="wqk")
            wvt = pp.tile([P, DT, HLOC * HD], BF16, tag="wv")
            wot = pp.tile([P, HLOC, D], BF16, tag="wo")
            cos_t = pp.tile([P, S], F32, tag="cos")
            sin_t = pp.tile([P, S], F32, tag="sin")

            nc.vector.memset(ones[:], 1.0)

            def o_proj(ic, aout):
                for st in range(CHUNK // P):
                    for dc in range(D // CHUNK):
                        ps = psA.tile([P, CHUNK], F32, tag="psA")
                        for kt in range(HLOC):
                            nc.tensor.matmul(
                                ps[:], aout[:, kt, P * st:P * (st + 1)],
                                wot[:, kt, CHUNK * dc:CHUNK * (dc + 1)],
                                start=(kt == 0), stop=(kt == HLOC - 1))
                        osb = op_.tile([P, CHUNK], F32, tag="osb")
                        nc.scalar.copy(osb[:], ps[:])
                        nc.scalar.dma_start(
                            o_r[:, ic * (CHUNK // P) + st,
                                CHUNK * dc:CHUNK * (dc + 1)], osb[:])

            def rope_store(ps_pe, dst_pe, cos_c, sin_c):
                """ps_pe: [64, CHUNK] psum AP at partition base 64 (pre-rope pe
                rows of one head). 4 DVE ops; sign-baked sin tables make the
                NeoX rotation a mult/mult/mult/add. dst_pe = rows [64:128]."""
                b = 64
                tmp = rp.tile([P, CHUNK], F32, tag="ropetmp")
                scr = rp.tile([P, CHUNK], F32, tag="ropescr")
                nc.vector.tensor_tensor(tmp[b:b + 32, :], ps_pe[32:64, :],
                                        sin_c[b:b + 32, :], mybir.AluOpType.mult)
                nc.vector.tensor_tensor(tmp[b + 32:b + 64, :], ps_pe[0:32, :],
                                        sin_c[b + 32:b + 64, :],
                                        mybir.AluOpType.mult)
                nc.vector.tensor_tensor(scr[b:b + 64, :], ps_pe[:],
                                        cos_c[b:b + 64, :], mybir.AluOpType.mult)
                nc.vector.tensor_tensor(dst_pe, scr[b:b + 64, :],
                                        tmp[b:b + 64, :], mybir.AluOpType.add)

            # ---------------- chunk loop ----------------
            for ic in range(NCHUNK):
                sl = slice(ic * CHUNK, (ic + 1) * CHUNK)

                xc = ap_.tile([P, DT, CHUNK], BF16, tag="xc")
                nc.sync.dma_start(xc[:], xt_r[:, ic])

                if ic == 0:
                    for ct in range(NCT):
                        nc.sync.dma_start(wqkt[:, ct], wqk_r[:, ct])
                    nc.scalar.dma_start(cos_t[:], cosr[:])
                    nc.scalar.dma_start(sin_t[:], sinr[:])
                    nc.scalar.dma_start(masks[:], mask_r[:])
                    nc.sync.dma_start(wvt[:], wv_r[:])

                cos_c = cos_t[:, sl]
                sin_c = sin_t[:, sl]

                # ---- projection: c-tile ct = head [nope64 | pe64] ----
                # ct 0..3 -> q heads, ct 4..7 -> k heads (identical rope)
                qTi = ap_.tile([P, HLOC, CHUNK], BF16, tag="qTi")
                for ct in range(NCT):
                    ps = psA.tile([P, CHUNK], F32, tag="psA")
                    for dt_ in range(DT):
                        nc.tensor.matmul(
                            ps[:], wqkt[:, ct, dt_, :], xc[:, dt_, :],
                            start=(dt_ == 0), stop=(dt_ == DT - 1))
                    if ct < HLOC:
                        dst_nope = qTi[0:64, ct, :]
                        dst_pe = qTi[64:128, ct, :]
                    else:
                        h = ct - HLOC
                        dst_nope = kT[0:64, h, sl]
                        dst_pe = kT[64:128, h, sl]
                    nc.vector.tensor_copy(dst_nope, ps[0:64, :])
                    rope_store(ps[64:128, :], dst_pe, cos_c, sin_c)

                if ic == 0:
                    # resident Wo load, deferred so it doesn't crowd the
                    # critical first-chunk x/weight DMAs
                    for kt in range(HLOC):
                        nc.sync.dma_start(wot[:, kt], wo_r[:, kt])

                # ---- v natural [CHUNK, 512]: x seq-tile stationary ----
                for st in range(CHUNK // P):
                    ps = psA.tile([P, HLOC * HD], F32, tag="psA")
                    for dt_ in range(DT):
                        nc.tensor.matmul(
                            ps[:], xc[:, dt_, P * st:P * (st + 1)],
                            wvt[:, dt_, :],
                            start=(dt_ == 0), stop=(dt_ == DT - 1))
                    nc.vector.tensor_copy(vnat[:, ic * (CHUNK // P) + st, :],
                                          ps[:])

                # ---- o-projection of the PREVIOUS chunk: PE work to cover
                # the DVE rope/normalize backlog of this chunk's projections
                if ic > 0:
                    o_proj(ic - 1, prev_aout)

                # ---- attention for this query chunk ----
                aout = aop.tile([P, HLOC, CHUNK], BF16, tag="aout")
                jt_max = (ic + 1) * (CHUNK // P)
                for h in range(HLOC):
                    psd = psD.tile([P, CHUNK], F32, tag="psD")
                    pso = psO.tile([P, CHUNK], F32, tag="psO")
                    for jt in range(jt_max):
                        pss = psS.tile([P, CHUNK], F32, tag="psS")
                        nc.tensor.matmul(
                            pss[:], kT[:, h, P * jt:P * (jt + 1)], qTi[:, h, :],
                            start=True, stop=True)
                        at = atp.tile([P, CHUNK], BF16, tag="attnT")
                        nc.scalar.activation(
                            at[:], pss[:], mybir.ActivationFunctionType.Exp,
                            scale=SCALE)
                        r = jt - ic * (CHUNK // P)
                        if r >= 0:  # diagonal tile: causal mask
                            nc.vector.tensor_tensor(
                                at[:], at[:], masks[:, r, :],
                                mybir.AluOpType.mult)
                        nc.tensor.matmul(psd[:], ones[:], at[:],
                                         start=(jt == 0), stop=(jt == jt_max - 1))
                        nc.tensor.matmul(
                            pso[:], vnat[:, jt, HD * h:HD * (h + 1)], at[:],
                            start=(jt == 0), stop=(jt == jt_max - 1))
                    rec = atp.tile([P, CHUNK], F32, tag="recip")
                    nc.vector.reciprocal_approx_fast(rec[:], psd[:])
                    nc.vector.tensor_tensor(aout[:, h, :], pso[:], rec[:],
                                            mybir.AluOpType.mult)
                prev_aout = aout

            o_proj(NCHUNK - 1, prev_aout)
    nc.compile()
    return nc


_NC = None


def _get_nc():
    global _NC
    if _NC is None:
        _NC = _build()
    return _NC


def _host_prep(x, Wq_down, Wq_up, Wq_rope, Wkv_down, Wk_up, Wk_rope, Wv_up, Wo):
    """Build the 8 per-core input maps (all host-side layout prep)."""
    # rope tables, replicated to 128 partitions with NeoX sign baked into sin
    half = RD // 2
    inv_freq = 1.0 / (BASE ** (np.arange(half, dtype=np.float64) / half))
    ang = np.arange(S, dtype=np.float64)[None, :] * inv_freq[:, None]  # [32, S]
    cos32 = np.cos(ang)
    sin32 = np.sin(ang)
    cosr = np.tile(cos32, (4, 1)).astype(np.float32)                   # [128,S]
    sinr = np.concatenate([-sin32, sin32, -sin32, sin32], 0).astype(np.float32)

    # causal masks for diagonal (j-tile, i-chunk) pairs: r = 0..3
    pidx = np.arange(P)[:, None]
    iidx = np.arange(CHUNK)[None, :]
    masks = np.stack([(pidx + P * r <= iidx) for r in range(4)], axis=1)
    maskd = masks.astype(_BF16).reshape(P, 4 * CHUNK)

    # fuse the low-rank compositions once, in f32
    Wfq = Wq_down @ Wq_up        # [D, H*ND]
    Wfqr = Wq_down @ Wq_rope     # [D, H*RD]
    Wfk = Wkv_down @ Wk_up       # [D, H*ND]
    Wfv = Wkv_down @ Wv_up       # [D, H*HD]

    # per-batch pre-tiled x^T: [p, ic, dt, s]
    xts = [np.ascontiguousarray(
        x[b].reshape(NCHUNK, CHUNK, DT, P).transpose(3, 0, 2, 1)
    ).reshape(P, -1).astype(_BF16) for b in range(B)]

    in_maps = []
    for c in range(8):
        b, g = divmod(c, 4)
        heads = range(HLOC * g, HLOC * (g + 1))
        Wqk = np.empty((D, NCT * P), np.float32)
        for i, h in enumerate(heads):
            q0, k0 = i * HD, HLOC * HD + i * HD
            Wqk[:, q0:q0 + ND] = Wfq[:, h * ND:(h + 1) * ND]
            Wqk[:, q0 + ND:q0 + HD] = Wfqr[:, h * RD:(h + 1) * RD]
            Wqk[:, k0:k0 + ND] = Wfk[:, h * ND:(h + 1) * ND]
            Wqk[:, k0 + ND:k0 + HD] = Wk_rope[:, h * RD:(h + 1) * RD]
        Wv = Wfv[:, g * HLOC * HD:(g + 1) * HLOC * HD]
        Wop = Wo[g * HLOC * HD:(g + 1) * HLOC * HD, :]
        in_maps.append({
            "xt": xts[b],
            "wqk": np.ascontiguousarray(
                Wqk.reshape(DT, P, NCT, P).transpose(1, 2, 0, 3)
            ).reshape(P, -1).astype(_BF16),
            "wv": np.ascontiguousarray(
                Wv.reshape(DT, P, HLOC * HD).transpose(1, 0, 2)
            ).reshape(P, -1).astype(_BF16),
            "wo": np.ascontiguousarray(
                Wop.reshape(HLOC, P, D).transpose(1, 0, 2)
            ).reshape(P, -1).astype(_BF16),
            "cosr": cosr,
            "sinr": sinr,
            "maskd": maskd,
        })
    return in_maps


def kernel(x, Wq_down, Wq_up, Wq_rope, Wkv_down, Wk_up, Wk_rope, Wv_up, Wo,
           _trace=False, _trace_kwargs=None):
    x = np.asarray(x, dtype=np.float32)
    args = [np.asarray(a, dtype=np.float32) for a in
            (Wq_down, Wq_up, Wq_rope, Wkv_down, Wk_up, Wk_rope, Wv_up, Wo)]
    in_maps = _host_prep(x, *args)
    nc = _get_nc()
    res = run_bass_kernel_spmd(nc, in_maps, core_ids=list(range(8)),
                               trace=_trace, **(_trace_kwargs or {}))
    kernel._last_results = res
    out = np.zeros((B, S, D), np.float32)
    for c in range(8):
        out[c // 4] += res.results[c]["o_part"]
    return out


# revision 7
# speedup vs baseline: 1.4313x; 1.0720x over previous
"""MLA attention (DeepSeek-style) Trainium2 Bass kernel, 8-core SPMD.

Sharding: core c handles batch b = c//4 and head-group g = c%4 (4 of 16 heads).
All low-rank projections are fused on the host (Wq_down@Wq_up etc.), so every
core runs a single head-parallel projection x @ Wqk [D, 1024] (per-head
[q_nope|q_pe|k_nope|k_pe] columns) + x @ Wv [D, 512] with ZERO replicated
work, then causal flash attention for its 4 heads and a partial o-projection.
Host sums the 4 partial o-projections per batch.

Device dataflow (per core, transposed-activation layout, S processed in 4
chunks of 512):
  xT (host-tiled, bf16) -> per-head qT/kT [128=HD, S] bf16 tiles straight from
  PSUM (nope rows 0:64, rope rows 64:128), RoPE via host-baked cos/sin tables
  -> V in natural layout via swapped-operand matmuls -> causal flash attention
  per head: scoresT [j,i] matmuls, exp on ScalarE (scale fused), unnormalized
  attnout + ones-matmul row sums, normalize by broadcast reciprocal ->
  o-projection -> partial [S, D] f32 out.

All DRAM tensors are host-pre-tiled to [128, ...] partition-major layout so
every DMA moves >=4KB contiguous per partition (fat descriptors).
"""

import numpy as np
import ml_dtypes

import concourse.bacc as bacc
import concourse.mybir as mybir
import concourse.tile as tile
from concourse.bass_utils import run_bass_kernel_spmd

F32 = mybir.dt.float32
BF16 = mybir.dt.bfloat16

B, S, D = 2, 2048, 2048
H, HD = 16, 128
RD, ND = 64, 64
KVR, QR = 512, 1024
BASE = 10000.0
HLOC = 4                 # heads per core
CHUNK = 512
NCHUNK = S // CHUNK      # 4
P = 128
DT = D // P              # 16 contraction tiles
NCT = 2 * HLOC           # 8 projection c-tiles (4 q heads + 4 k heads)
SCALE = HD ** -0.5

_BF16 = ml_dtypes.bfloat16


def _build():
    nc = bacc.Bacc("TRN2", target_bir_lowering=False, debug=False)

    # pre-tiled layouts: free index order is given in the rearrange below
    xt = nc.dram_tensor("xt", [P, NCHUNK * DT * CHUNK], BF16,
                        kind="ExternalInput").ap()
    wqk = nc.dram_tensor("wqk", [P, NCT * DT * P], BF16,
                         kind="ExternalInput").ap()
    wv = nc.dram_tensor("wv", [P, DT * HLOC * HD], BF16,
                        kind="ExternalInput").ap()
    wo = nc.dram_tensor("wo", [P, HLOC * D], BF16, kind="ExternalInput").ap()
    cosr = nc.dram_tensor("cosr", [P, S], F32, kind="ExternalInput").ap()
    sinr = nc.dram_tensor("sinr", [P, S], F32, kind="ExternalInput").ap()
    maskd = nc.dram_tensor("maskd", [P, 4 * CHUNK], BF16,
                           kind="ExternalInput").ap()
    o_part = nc.dram_tensor("o_part", [S, D], BF16, kind="ExternalOutput").ap()

    xt_r = xt.rearrange("p (ic dt s) -> p ic dt s", ic=NCHUNK, dt=DT)
    wqk_r = wqk.rearrange("p (ct dt c) -> p ct dt c", ct=NCT, dt=DT)
    wv_r = wv.rearrange("p (dt c) -> p dt c", dt=DT)
    wo_r = wo.rearrange("p (kt d) -> p kt d", kt=HLOC)
    mask_r = maskd.rearrange("p (r i) -> p r i", r=4)
    o_r = o_part.rearrange("(st p) d -> p st d", p=P)       # [128, 16, 2048]

    with tile.TileContext(nc) as tc:
        with (
            tc.tile_pool(name="persist", bufs=1) as pp,
            tc.tile_pool(name="acts", bufs=2) as ap_,
            tc.tile_pool(name="rope", bufs=2) as rp,
            tc.tile_pool(name="attn", bufs=2) as atp,
            tc.tile_pool(name="outp", bufs=4) as op_,
            tc.tile_pool(name="aoutp", bufs=2) as aop,
            tc.tile_pool(name="psA", bufs=2, space="PSUM") as psA,
            tc.tile_pool(name="psS", bufs=2, space="PSUM") as psS,
            tc.tile_pool(name="psD", bufs=2, space="PSUM") as psD,
            tc.tile_pool(name="psO", bufs=2, space="PSUM") as psO,
        ):
            # ---------------- persistent tiles ----------------
            kT = pp.tile([P, HLOC, S], BF16, tag="kT")            # per-head K^T
            vnat = pp.tile([P, S // P, HLOC * HD], BF16, tag="vnat")  # V natural
            masks = pp.tile([P, 4, CHUNK], BF16, tag="masks")
            ones = pp.tile([P, P], BF16, tag="ones")
            wqkt = pp.tile([P, NCT, DT, P], BF16, tag="wqk")
            wvt = pp.tile([P, DT, HLOC * HD], BF16, tag="wv")
            wot = pp.tile([P, HLOC, D], BF16, tag="wo")
            cos_t = pp.tile([P, S], F32, tag="cos")
            sin_t = pp.tile([P, S], F32, tag="sin")

            nc.vector.memset(ones[:], 1.0)

            def o_proj(ic, aout):
                for st in range(CHUNK // P):
                    for dc in range(D // CHUNK):
                        ps = psA.tile([P, CHUNK], F32, tag="psA")
                        for kt in range(HLOC):
                            nc.tensor.matmul(
                                ps[:], aout[:, kt, P * st:P * (st + 1)],
                                wot[:, kt, CHUNK * dc:CHUNK * (dc + 1)],
                                start=(kt == 0), stop=(kt == HLOC - 1))
                        osb = op_.tile([P, CHUNK], BF16, tag="osb")
                        nc.vector.tensor_copy(osb[:], ps[:])
                        nc.scalar.dma_start(
                            o_r[:, ic * (CHUNK // P) + st,
                                CHUNK * dc:CHUNK * (dc + 1)], osb[:])

            def rope_store(ps_pe, dst_pe, cos_c, sin_c):
                """ps_pe: [64, CHUNK] psum AP at partition base 64 (pre-rope pe
                rows of one head). 4 DVE ops; sign-baked sin tables make the
                NeoX rotation a mult/mult/mult/add. dst_pe = rows [64:128]."""
                b = 64
                tmp = rp.tile([P, CHUNK], F32, tag="ropetmp")
                scr = rp.tile([P, CHUNK], F32, tag="ropescr")
                nc.vector.tensor_tensor(tmp[b:b + 32, :], ps_pe[32:64, :],
                                        sin_c[b:b + 32, :], mybir.AluOpType.mult)
                nc.vector.tensor_tensor(tmp[b + 32:b + 64, :], ps_pe[0:32, :],
                                        sin_c[b + 32:b + 64, :],
                                        mybir.AluOpType.mult)
                nc.vector.tensor_tensor(scr[b:b + 64, :], ps_pe[:],
                                        cos_c[b:b + 64, :], mybir.AluOpType.mult)
                nc.vector.tensor_tensor(dst_pe, scr[b:b + 64, :],
                                        tmp[b:b + 64, :], mybir.AluOpType.add)

            # ---------------- chunk loop ----------------
            for ic in range(NCHUNK):
                sl = slice(ic * CHUNK, (ic + 1) * CHUNK)

                xc = ap_.tile([P, DT, CHUNK], BF16, tag="xc")
                for qd in range(4):  # parallel DMA queues
                    nc.sync.dma_start(xc[:, 4 * qd:4 * (qd + 1)],
                                      xt_r[:, ic, 4 * qd:4 * (qd + 1)])

                if ic == 0:
                    for ct in range(NCT):
                        nc.sync.dma_start(wqkt[:, ct], wqk_r[:, ct])
                    nc.scalar.dma_start(cos_t[:], cosr[:])
                    nc.scalar.dma_start(sin_t[:], sinr[:])
                    nc.scalar.dma_start(masks[:], mask_r[:])
                    for qd in range(4):
                        nc.sync.dma_start(wvt[:, 4 * qd:4 * (qd + 1)],
                                          wv_r[:, 4 * qd:4 * (qd + 1)])

                cos_c = cos_t[:, sl]
                sin_c = sin_t[:, sl]

                # ---- projection: c-tile ct = head [nope64 | pe64] ----
                # ct 0..3 -> q heads, ct 4..7 -> k heads (identical rope)
                qTi = ap_.tile([P, HLOC, CHUNK], BF16, tag="qTi")
                for ct in range(NCT):
                    ps = psA.tile([P, CHUNK], F32, tag="psA")
                    for dt_ in range(DT):
                        nc.tensor.matmul(
                            ps[:], wqkt[:, ct, dt_, :], xc[:, dt_, :],
                            start=(dt_ == 0), stop=(dt_ == DT - 1))
                    if ct < HLOC:
                        dst_nope = qTi[0:64, ct, :]
                        dst_pe = qTi[64:128, ct, :]
                    else:
                        h = ct - HLOC
                        dst_nope = kT[0:64, h, sl]
                        dst_pe = kT[64:128, h, sl]
                    nc.vector.tensor_copy(dst_nope, ps[0:64, :])
                    rope_store(ps[64:128, :], dst_pe, cos_c, sin_c)

                if ic == 0:
                    # resident Wo load, deferred so it doesn't crowd the
                    # critical first-chunk x/weight DMAs
                    for kt in range(HLOC):
                        nc.sync.dma_start(wot[:, kt], wo_r[:, kt])

                # ---- v natural [CHUNK, 512]: x seq-tile stationary ----
                for st in range(CHUNK // P):
                    ps = psA.tile([P, HLOC * HD], F32, tag="psA")
                    for dt_ in range(DT):
                        nc.tensor.matmul(
                            ps[:], xc[:, dt_, P * st:P * (st + 1)],
                            wvt[:, dt_, :],
                            start=(dt_ == 0), stop=(dt_ == DT - 1))
                    nc.vector.tensor_copy(vnat[:, ic * (CHUNK // P) + st, :],
                                          ps[:])

                # ---- o-projection of the PREVIOUS chunk: PE work to cover
                # the DVE rope/normalize backlog of this chunk's projections
                if ic > 0:
                    o_proj(ic - 1, prev_aout)

                # ---- attention for this query chunk ----
                aout = aop.tile([P, HLOC, CHUNK], BF16, tag="aout")
                jt_max = (ic + 1) * (CHUNK // P)
                for h in range(HLOC):
                    psd = psD.tile([P, CHUNK], F32, tag="psD")
                    pso = psO.tile([P, CHUNK], F32, tag="psO")
                    for jt in range(jt_max):
                        pss = psS.tile([P, CHUNK], F32, tag="psS")
                        nc.tensor.matmul(
                            pss[:], kT[:, h, P * jt:P * (jt + 1)], qTi[:, h, :],
                            start=True, stop=True)
                        at = atp.tile([P, CHUNK], BF16, tag="attnT")
                        nc.scalar.activation(
                            at[:], pss[:], mybir.ActivationFunctionType.Exp,
                            scale=SCALE)
                        r = jt - ic * (CHUNK // P)
                        if r >= 0:  # diagonal tile: causal mask
                            nc.vector.tensor_tensor(
                                at[:], at[:], masks[:, r, :],
                                mybir.AluOpType.mult)
                        nc.tensor.matmul(psd[:], ones[:], at[:],
                                         start=(jt == 0), stop=(jt == jt_max - 1))
                        nc.tensor.matmul(
                            pso[:], vnat[:, jt, HD * h:HD * (h + 1)], at[:],
                            start=(jt == 0), stop=(jt == jt_max - 1))
                    rec = atp.tile([P, CHUNK], F32, tag="recip")
                    nc.vector.reciprocal_approx_fast(rec[:], psd[:])
                    nc.vector.tensor_tensor(aout[:, h, :], pso[:], rec[:],
                                            mybir.AluOpType.mult)
                prev_aout = aout

            o_proj(NCHUNK - 1, prev_aout)
    nc.compile()
    return nc


_NC = None


def _get_nc():
    global _NC
    if _NC is None:
        _NC = _build()
    return _NC


def _host_prep(x, Wq_down, Wq_up, Wq_rope, Wkv_down, Wk_up, Wk_rope, Wv_up, Wo):
    """Build the 8 per-core input maps (all host-side layout prep)."""
    # rope tables, replicated to 128 partitions with NeoX sign baked into sin
    half = RD // 2
    inv_freq = 1.0 / (BASE ** (np.arange(half, dtype=np.float64) / half))
    ang = np.arange(S, dtype=np.float64)[None, :] * inv_freq[:, None]  # [32, S]
    cos32 = np.cos(ang)
    sin32 = np.sin(ang)
    cosr = np.tile(cos32, (4, 1)).astype(np.float32)                   # [128,S]
    sinr = np.concatenate([-sin32, sin32, -sin32, sin32], 0).astype(np.float32)

    # causal masks for diagonal (j-tile, i-chunk) pairs: r = 0..3
    pidx = np.arange(P)[:, None]
    iidx = np.arange(CHUNK)[None, :]
    masks = np.stack([(pidx + P * r <= iidx) for r in range(4)], axis=1)
    maskd = masks.astype(_BF16).reshape(P, 4 * CHUNK)

    # fuse the low-rank compositions once, in f32
    Wfq = Wq_down @ Wq_up        # [D, H*ND]
    Wfqr = Wq_down @ Wq_rope     # [D, H*RD]
    Wfk = Wkv_down @ Wk_up       # [D, H*ND]
    Wfv = Wkv_down @ Wv_up       # [D, H*HD]

    # per-batch pre-tiled x^T: [p, ic, dt, s]
    xts = [np.ascontiguousarray(
        x[b].reshape(NCHUNK, CHUNK, DT, P).transpose(3, 0, 2, 1)
    ).reshape(P, -1).astype(_BF16) for b in range(B)]

    in_maps = []
    for c in range(8):
        b, g = divmod(c, 4)
        heads = range(HLOC * g, HLOC * (g + 1))
        Wqk = np.empty((D, NCT * P), np.float32)
        for i, h in enumerate(heads):
            q0, k0 = i * HD, HLOC * HD + i * HD
            Wqk[:, q0:q0 + ND] = Wfq[:, h * ND:(h + 1) * ND]
            Wqk[:, q0 + ND:q0 + HD] = Wfqr[:, h * RD:(h + 1) * RD]
            Wqk[:, k0:k0 + ND] = Wfk[:, h * ND:(h + 1) * ND]
            Wqk[:, k0 + ND:k0 + HD] = Wk_rope[:, h * RD:(h + 1) * RD]
        Wv = Wfv[:, g * HLOC * HD:(g + 1) * HLOC * HD]
        Wop = Wo[g * HLOC * HD:(g + 1) * HLOC * HD, :]
        in_maps.append({
            "xt": xts[b],
            "wqk": np.ascontiguousarray(
                Wqk.reshape(DT, P, NCT, P).transpose(1, 2, 0, 3)
            ).reshape(P, -1).astype(_BF16),
            "wv": np.ascontiguousarray(
                Wv.reshape(DT, P, HLOC * HD).transpose(1, 0, 2)
            ).reshape(P, -1).astype(_BF16),
            "wo": np.ascontiguousarray(
                Wop.reshape(HLOC, P, D).transpose(1, 0, 2)
            ).reshape(P, -1).astype(_BF16),
            "cosr": cosr,
            "sinr": sinr,
            "maskd": maskd,
        })
    return in_maps


def kernel(x, Wq_down, Wq_up, Wq_rope, Wkv_down, Wk_up, Wk_rope, Wv_up, Wo,
           _trace=False, _trace_kwargs=None):
    x = np.asarray(x, dtype=np.float32)
    args = [np.asarray(a, dtype=np.float32) for a in
            (Wq_down, Wq_up, Wq_rope, Wkv_down, Wk_up, Wk_rope, Wv_up, Wo)]
    in_maps = _host_prep(x, *args)
    nc = _get_nc()
    res = run_bass_kernel_spmd(nc, in_maps, core_ids=list(range(8)),
                               trace=_trace, **(_trace_kwargs or {}))
    kernel._last_results = res
    out = np.zeros((B, S, D), np.float32)
    for c in range(8):
        out[c // 4] += res.results[c]["o_part"].astype(np.float32)
    return out


# revision 12
# speedup vs baseline: 1.4459x; 1.0102x over previous
"""MLA attention (DeepSeek-style) Trainium2 Bass kernel, 8-core SPMD.

Sharding: core c handles batch b = c//4 and head-group g = c%4 (4 of 16 heads).
All low-rank projections are fused on the host (Wq_down@Wq_up etc.), so every
core runs a single head-parallel projection x @ Wqk [D, 1024] (per-head
[q_nope|q_pe|k_nope|k_pe] columns) + x @ Wv [D, 512] with ZERO replicated
work, then causal flash attention for its 4 heads and a partial o-projection.
Host sums the 4 partial o-projections per batch.

Device dataflow (per core, transposed-activation layout, S processed in 4
chunks of 512):
  xT (host-tiled, bf16) -> per-head qT/kT [128=HD, S] bf16 tiles straight from
  PSUM (nope rows 0:64, rope rows 64:128), RoPE via host-baked cos/sin tables
  -> V in natural layout via swapped-operand matmuls -> causal flash attention
  per head: scoresT [j,i] matmuls, exp on ScalarE (scale fused), unnormalized
  attnout + ones-matmul row sums, normalize by broadcast reciprocal ->
  o-projection -> partial [S, D] f32 out.

All DRAM tensors are host-pre-tiled to [128, ...] partition-major layout so
every DMA moves >=4KB contiguous per partition (fat descriptors).
"""

import numpy as np
import ml_dtypes

import concourse.bacc as bacc
import concourse.mybir as mybir
import concourse.tile as tile
from concourse.bass_utils import run_bass_kernel_spmd

F32 = mybir.dt.float32
BF16 = mybir.dt.bfloat16

B, S, D = 2, 2048, 2048
H, HD = 16, 128
RD, ND = 64, 64
KVR, QR = 512, 1024
BASE = 10000.0
HLOC = 4                 # heads per core
CHUNK = 512
NCHUNK = S // CHUNK      # 4
P = 128
DT = D // P              # 16 contraction tiles
NCT = 2 * HLOC           # 8 projection c-tiles (4 q heads + 4 k heads)
SCALE = HD ** -0.5

_BF16 = ml_dtypes.bfloat16


def _build():
    nc = bacc.Bacc("TRN2", target_bir_lowering=False, debug=False)

    # pre-tiled layouts: free index order is given in the rearrange below
    xt = nc.dram_tensor("xt", [P, NCHUNK * DT * CHUNK], BF16,
                        kind="ExternalInput").ap()
    wqk = nc.dram_tensor("wqk", [P, NCT * DT * P], BF16,
                         kind="ExternalInput").ap()
    wv = nc.dram_tensor("wv", [P, DT * HLOC * HD], BF16,
                        kind="ExternalInput").ap()
    wo = nc.dram_tensor("wo", [P, HLOC * D], BF16, kind="ExternalInput").ap()
    cosr = nc.dram_tensor("cosr", [RD, S], F32, kind="ExternalInput").ap()
    sinr = nc.dram_tensor("sinr", [RD, S], F32, kind="ExternalInput").ap()
    maskd = nc.dram_tensor("maskd", [P, 4 * CHUNK], BF16,
                           kind="ExternalInput").ap()
    # output pre-tiled [p, st, d]: fat 4KB-per-partition DMA descriptors
    o_part = nc.dram_tensor("o_part", [P, (S // P) * D], BF16,
                            kind="ExternalOutput").ap()

    xt_r = xt.rearrange("p (ic dt s) -> p ic dt s", ic=NCHUNK, dt=DT)
    wqk_r = wqk.rearrange("p (ct dt c) -> p ct dt c", ct=NCT, dt=DT)
    wv_r = wv.rearrange("p (dt c) -> p dt c", dt=DT)
    wo_r = wo.rearrange("p (kt d) -> p kt d", kt=HLOC)
    mask_r = maskd.rearrange("p (r i) -> p r i", r=4)
    o_r = o_part.rearrange("p (st d) -> p st d", st=S // P)  # [128, 16, 2048]

    with tile.TileContext(nc) as tc:
        with (
            tc.tile_pool(name="persist", bufs=1) as pp,
            tc.tile_pool(name="acts", bufs=2) as ap_,
            tc.tile_pool(name="rope", bufs=2) as rp,
            tc.tile_pool(name="attn", bufs=2) as atp,
            tc.tile_pool(name="outp", bufs=4) as op_,
            tc.tile_pool(name="aoutp", bufs=2) as aop,
            tc.tile_pool(name="psA", bufs=2, space="PSUM") as psA,
            tc.tile_pool(name="psS", bufs=2, space="PSUM") as psS,
            tc.tile_pool(name="psD", bufs=2, space="PSUM") as psD,
            tc.tile_pool(name="psO", bufs=2, space="PSUM") as psO,
        ):
            # ---------------- persistent tiles ----------------
            kT = pp.tile([P, HLOC, S], BF16, tag="kT")            # per-head K^T
            vnat = pp.tile([P, S // P, HLOC * HD], BF16, tag="vnat")  # V natural
            masks = pp.tile([P, 4, CHUNK], BF16, tag="masks")
            ones = pp.tile([P, P], BF16, tag="ones")
            wqkt = pp.tile([P, NCT, DT, P], BF16, tag="wqk")
            wvt = pp.tile([P, DT, HLOC * HD], BF16, tag="wv")
            wot = pp.tile([P, HLOC, D], BF16, tag="wo")
            cos_t = pp.tile([P, S], F32, tag="cos")
            sin_t = pp.tile([P, S], F32, tag="sin")

            nc.vector.memset(ones[:], 1.0)

            def o_proj(ic, aout):
                for st in range(CHUNK // P):
                    osb = op_.tile([P, D // CHUNK, CHUNK], BF16, tag="osb")
                    for dc in range(D // CHUNK):
                        ps = psA.tile([P, CHUNK], F32, tag="psA")
                        for kt in range(HLOC):
                            nc.tensor.matmul(
                                ps[:], aout[:, kt, P * st:P * (st + 1)],
                                wot[:, kt, CHUNK * dc:CHUNK * (dc + 1)],
                                start=(kt == 0), stop=(kt == HLOC - 1))
                        if dc % 2 == 0:
                            nc.vector.tensor_copy(osb[:, dc, :], ps[:])
                        else:
                            nc.scalar.copy(osb[:, dc, :], ps[:])
                    nc.scalar.dma_start(
                        o_r[:, ic * (CHUNK // P) + st, :], osb[:])

            def rope_store(ps_pe, dst_pe, cos_c, sin_c):
                """ps_pe: [64, CHUNK] psum AP at partition base 64 (pre-rope pe
                rows of one head). 4 DVE ops; sign-baked sin tables make the
                NeoX rotation a mult/mult/mult/add. dst_pe = rows [64:128]."""
                b = 64
                tmp = rp.tile([P, CHUNK], F32, tag="ropetmp")
                scr = rp.tile([P, CHUNK], F32, tag="ropescr")
                nc.vector.tensor_tensor(tmp[b:b + 32, :], ps_pe[32:64, :],
                                        sin_c[b:b + 32, :], mybir.AluOpType.mult)
                nc.vector.tensor_tensor(tmp[b + 32:b + 64, :], ps_pe[0:32, :],
                                        sin_c[b + 32:b + 64, :],
                                        mybir.AluOpType.mult)
                nc.vector.tensor_tensor(scr[b:b + 64, :], ps_pe[:],
                                        cos_c[b:b + 64, :], mybir.AluOpType.mult)
                nc.vector.tensor_tensor(dst_pe, scr[b:b + 64, :],
                                        tmp[b:b + 64, :], mybir.AluOpType.add)

            # ---------------- chunk loop ----------------
            for ic in range(NCHUNK):
                sl = slice(ic * CHUNK, (ic + 1) * CHUNK)

                xc = ap_.tile([P, DT, CHUNK], BF16, tag="xc")
                for qd in range(4):  # parallel DMA queues
                    nc.sync.dma_start(xc[:, 4 * qd:4 * (qd + 1)],
                                      xt_r[:, ic, 4 * qd:4 * (qd + 1)])

                if ic == 0:
                    # critical-path loads first: c-tiles 0-1 + rope tables,
                    # then the rest (needed progressively later)
                    nc.sync.dma_start(wqkt[:, 0], wqk_r[:, 0])
                    nc.sync.dma_start(wqkt[:, 1], wqk_r[:, 1])
                    nc.scalar.dma_start(cos_t[64:128, :], cosr[:])
                    nc.scalar.dma_start(sin_t[64:128, :], sinr[:])
                    for ct in range(2, NCT):
                        nc.sync.dma_start(wqkt[:, ct], wqk_r[:, ct])
                    nc.scalar.dma_start(masks[:], mask_r[:])
                    for qd in range(4):
                        nc.sync.dma_start(wvt[:, 4 * qd:4 * (qd + 1)],
                                          wv_r[:, 4 * qd:4 * (qd + 1)])

                cos_c = cos_t[:, sl]
                sin_c = sin_t[:, sl]

                # ---- projection: c-tile ct = head [nope64 | pe64] ----
                # ct 0..3 -> q heads, ct 4..7 -> k heads (identical rope)
                qTi = ap_.tile([P, HLOC, CHUNK], BF16, tag="qTi")
                for ct in range(NCT):
                    ps = psA.tile([P, CHUNK], F32, tag="psA")
                    for dt_ in range(DT):
                        nc.tensor.matmul(
                            ps[:], wqkt[:, ct, dt_, :], xc[:, dt_, :],
                            start=(dt_ == 0), stop=(dt_ == DT - 1))
                    if ct < HLOC:
                        dst_nope = qTi[0:64, ct, :]
                        dst_pe = qTi[64:128, ct, :]
                    else:
                        h = ct - HLOC
                        dst_nope = kT[0:64, h, sl]
                        dst_pe = kT[64:128, h, sl]
                    nc.vector.tensor_copy(dst_nope, ps[0:64, :])
                    rope_store(ps[64:128, :], dst_pe, cos_c, sin_c)

                if ic == 0:
                    # resident Wo load, deferred so it doesn't crowd the
                    # critical first-chunk x/weight DMAs
                    for kt in range(HLOC):
                        nc.sync.dma_start(wot[:, kt], wo_r[:, kt])

                # ---- v natural [CHUNK, 512]: x seq-tile stationary ----
                for st in range(CHUNK // P):
                    ps = psA.tile([P, HLOC * HD], F32, tag="psA")
                    for dt_ in range(DT):
                        nc.tensor.matmul(
                            ps[:], xc[:, dt_, P * st:P * (st + 1)],
                            wvt[:, dt_, :],
                            start=(dt_ == 0), stop=(dt_ == DT - 1))
                    nc.vector.tensor_copy(vnat[:, ic * (CHUNK // P) + st, :],
                                          ps[:])

                # ---- o-projection of the PREVIOUS chunk: PE work to cover
                # the DVE rope/normalize backlog of this chunk's projections
                if ic > 0:
                    o_proj(ic - 1, prev_aout)

                # ---- attention for this query chunk ----
                aout = aop.tile([P, HLOC, CHUNK], BF16, tag="aout")
                jt_max = (ic + 1) * (CHUNK // P)
                for h in range(HLOC):
                    psd = psD.tile([P, CHUNK], F32, tag="psD")
                    pso = psO.tile([P, CHUNK], F32, tag="psO")
                    for jt in range(jt_max):
                        pss = psS.tile([P, CHUNK], F32, tag="psS")
                        nc.tensor.matmul(
                            pss[:], kT[:, h, P * jt:P * (jt + 1)], qTi[:, h, :],
                            start=True, stop=True)
                        at = atp.tile([P, CHUNK], BF16, tag="attnT")
                        nc.scalar.activation(
                            at[:], pss[:], mybir.ActivationFunctionType.Exp,
                            scale=SCALE)
                        r = jt - ic * (CHUNK // P)
                        if r >= 0:  # diagonal tile: causal mask
                            nc.vector.tensor_tensor(
                                at[:], at[:], masks[:, r, :],
                                mybir.AluOpType.mult)
                        nc.tensor.matmul(psd[:], ones[:], at[:],
                                         start=(jt == 0), stop=(jt == jt_max - 1))
                        nc.tensor.matmul(
                            pso[:], vnat[:, jt, HD * h:HD * (h + 1)], at[:],
                            start=(jt == 0), stop=(jt == jt_max - 1))
                    rec = atp.tile([P, CHUNK], F32, tag="recip")
                    nc.vector.reciprocal_approx_fast(rec[:], psd[:])
                    nc.vector.tensor_tensor(aout[:, h, :], pso[:], rec[:],
                                            mybir.AluOpType.mult)
                prev_aout = aout

            o_proj(NCHUNK - 1, prev_aout)
    nc.compile()
    return nc


_NC = None


def _get_nc():
    global _NC
    if _NC is None:
        _NC = _build()
    return _NC


def _host_prep(x, Wq_down, Wq_up, Wq_rope, Wkv_down, Wk_up, Wk_rope, Wv_up, Wo):
    """Build the 8 per-core input maps (all host-side layout prep)."""
    # rope tables, replicated to 128 partitions with NeoX sign baked into sin
    half = RD // 2
    inv_freq = 1.0 / (BASE ** (np.arange(half, dtype=np.float64) / half))
    ang = np.arange(S, dtype=np.float64)[None, :] * inv_freq[:, None]  # [32, S]
    cos32 = np.cos(ang)
    sin32 = np.sin(ang)
    # 64-row tables for SBUF partition rows 64:128 (the pe rows)
    cosr = np.tile(cos32, (2, 1)).astype(np.float32)                   # [64,S]
    sinr = np.concatenate([-sin32, sin32], 0).astype(np.float32)

    # causal masks for diagonal (j-tile, i-chunk) pairs: r = 0..3
    pidx = np.arange(P)[:, None]
    iidx = np.arange(CHUNK)[None, :]
    masks = np.stack([(pidx + P * r <= iidx) for r in range(4)], axis=1)
    maskd = masks.astype(_BF16).reshape(P, 4 * CHUNK)

    # fuse the low-rank compositions once, in f32
    Wfq = Wq_down @ Wq_up        # [D, H*ND]
    Wfqr = Wq_down @ Wq_rope     # [D, H*RD]
    Wfk = Wkv_down @ Wk_up       # [D, H*ND]
    Wfv = Wkv_down @ Wv_up       # [D, H*HD]

    # per-batch pre-tiled x^T: [p, ic, dt, s]
    xts = [np.ascontiguousarray(
        x[b].reshape(NCHUNK, CHUNK, DT, P).transpose(3, 0, 2, 1)
    ).reshape(P, -1).astype(_BF16) for b in range(B)]

    in_maps = []
    for c in range(8):
        b, g = divmod(c, 4)
        heads = range(HLOC * g, HLOC * (g + 1))
        Wqk = np.empty((D, NCT * P), np.float32)
        for i, h in enumerate(heads):
            q0, k0 = i * HD, HLOC * HD + i * HD
            Wqk[:, q0:q0 + ND] = Wfq[:, h * ND:(h + 1) * ND]
            Wqk[:, q0 + ND:q0 + HD] = Wfqr[:, h * RD:(h + 1) * RD]
            Wqk[:, k0:k0 + ND] = Wfk[:, h * ND:(h + 1) * ND]
            Wqk[:, k0 + ND:k0 + HD] = Wk_rope[:, h * RD:(h + 1) * RD]
        Wv = Wfv[:, g * HLOC * HD:(g + 1) * HLOC * HD]
        Wop = Wo[g * HLOC * HD:(g + 1) * HLOC * HD, :]
        in_maps.append({
            "xt": xts[b],
            "wqk": np.ascontiguousarray(
                Wqk.reshape(DT, P, NCT, P).transpose(1, 2, 0, 3)
            ).reshape(P, -1).astype(_BF16),
            "wv": np.ascontiguousarray(
                Wv.reshape(DT, P, HLOC * HD).transpose(1, 0, 2)
            ).reshape(P, -1).astype(_BF16),
            "wo": np.ascontiguousarray(
                Wop.reshape(HLOC, P, D).transpose(1, 0, 2)
            ).reshape(P, -1).astype(_BF16),
            "cosr": cosr,
            "sinr": sinr,
            "maskd": maskd,
        })
    return in_maps


def kernel(x, Wq_down, Wq_up, Wq_rope, Wkv_down, Wk_up, Wk_rope, Wv_up, Wo,
           _trace=False, _trace_kwargs=None):
    x = np.asarray(x, dtype=np.float32)
    args = [np.asarray(a, dtype=np.float32) for a in
            (Wq_down, Wq_up, Wq_rope, Wkv_down, Wk_up, Wk_rope, Wv_up, Wo)]
    in_maps = _host_prep(x, *args)
    nc = _get_nc()
    res = run_bass_kernel_spmd(nc, in_maps, core_ids=list(range(8)),
                               trace=_trace, **(_trace_kwargs or {}))
    kernel._last_results = res
    out = np.zeros((B, S, D), np.float32)
    for c in range(8):
        # un-tile [p, st, d] -> [st*128+p, d]
        part = res.results[c]["o_part"].reshape(P, S // P, D)
        out[c // 4] += part.transpose(1, 0, 2).reshape(S, D).astype(np.float32)
    return out


# revision 14
# speedup vs baseline: 1.5228x; 1.0532x over previous
"""MLA attention (DeepSeek-style) Trainium2 Bass kernel, 8-core SPMD.

Sharding: core c handles batch b = c//4 and head-group g = c%4 (4 of 16 heads).
All low-rank projections are fused on the host (Wq_down@Wq_up etc.), so every
core runs a single head-parallel projection x @ Wqk [D, 1024] (per-head
[q_nope|q_pe] / [k_nope|k_pe] column tiles) + x @ Wv [D, 512] with ZERO
replicated work, then causal flash attention for its 4 heads and a partial
o-projection. Host sums the 4 partial o-projections per batch.

Device dataflow (per core, transposed-activation layout, S processed in 4
chunks of 512):
  xT (host-tiled, bf16) -> per-head qT/kT [128=HD, S] bf16 tiles straight from
  PSUM (nope rows 0:64, rope rows 64:128), RoPE via host-baked cos/sin tables
  -> V in natural layout via swapped-operand matmuls -> causal flash attention
  per head: scoresT [j,i] matmuls with diagonal tiles shrunk to the unmasked
  query range, exp on ScalarE (scale fused), unnormalized attnout + ones-matmul
  row sums, normalize by broadcast reciprocal -> o-projection -> partial
  [S, D] bf16 out (pre-tiled layout).

Every DMA batch gets its own SBUF tile (dependency tracking is per-tile, so
shared tiles would serialize consumers on the LAST dma). Weights stream on the
gpsimd ring, x on the sync ring, tables/outputs on the scalar ring.
"""

import numpy as np
import ml_dtypes

import concourse.bacc as bacc
import concourse.mybir as mybir
import concourse.tile as tile
from concourse.bass_utils import run_bass_kernel_spmd

F32 = mybir.dt.float32
BF16 = mybir.dt.bfloat16

B, S, D = 2, 2048, 2048
H, HD = 16, 128
RD, ND = 64, 64
KVR, QR = 512, 1024
BASE = 10000.0
HLOC = 4                 # heads per core
CHUNK = 512
NCHUNK = S // CHUNK      # 4
P = 128
DT = D // P              # 16 contraction tiles
NCT = 2 * HLOC           # 8 projection c-tiles (4 q heads + 4 k heads)
SCALE = HD ** -0.5

_BF16 = ml_dtypes.bfloat16


def _build():
    nc = bacc.Bacc("TRN2", target_bir_lowering=False, debug=False)

    xt = nc.dram_tensor("xt", [P, NCHUNK * DT * CHUNK], BF16,
                        kind="ExternalInput").ap()
    wqk = nc.dram_tensor("wqk", [P, NCT * DT * P], BF16,
                         kind="ExternalInput").ap()
    wv = nc.dram_tensor("wv", [P, DT * HLOC * HD], BF16,
                        kind="ExternalInput").ap()
    wo = nc.dram_tensor("wo", [P, HLOC * D], BF16, kind="ExternalInput").ap()
    cosr = nc.dram_tensor("cosr", [RD, S], F32, kind="ExternalInput").ap()
    sinr = nc.dram_tensor("sinr", [RD, S], F32, kind="ExternalInput").ap()
    maskd = nc.dram_tensor("maskd", [P, P], BF16, kind="ExternalInput").ap()
    # output pre-tiled [p, st, d]: fat 4KB-per-partition DMA descriptors
    o_part = nc.dram_tensor("o_part", [P, (S // P) * D], BF16,
                            kind="ExternalOutput").ap()

    xt_r = xt.rearrange("p (ic hf dt s) -> p ic hf dt s",
                        ic=NCHUNK, hf=2, dt=DT // 2)
    wqk_r = wqk.rearrange("p (ct dt c) -> p ct dt c", ct=NCT, dt=DT)
    wv_r = wv.rearrange("p (hf dt c) -> p hf dt c", hf=2, dt=DT // 2)
    wo_r = wo.rearrange("p (kt d) -> p kt d", kt=HLOC)
    o_r = o_part.rearrange("p (st d) -> p st d", st=S // P)  # [128, 16, 2048]

    with tile.TileContext(nc) as tc:
        with (
            tc.tile_pool(name="persist", bufs=1) as pp,
            tc.tile_pool(name="acts", bufs=2) as ap_,
            tc.tile_pool(name="rope", bufs=2) as rp,
            tc.tile_pool(name="attn", bufs=2) as atp,
            tc.tile_pool(name="outp", bufs=2) as op_,
            tc.tile_pool(name="aoutp", bufs=2) as aop,
            tc.tile_pool(name="psA", bufs=2, space="PSUM") as psA,
            tc.tile_pool(name="psS", bufs=2, space="PSUM") as psS,
            tc.tile_pool(name="psD", bufs=2, space="PSUM") as psD,
            tc.tile_pool(name="psO", bufs=2, space="PSUM") as psO,
        ):
            # ---------------- persistent tiles (one per DMA batch) ----------
            kTs = [pp.tile([P, S], BF16, tag=f"kT{h}", name=f"kT{h}")
                   for h in range(HLOC)]
            vnat = pp.tile([P, S // P, HLOC * HD], BF16, tag="vnat")
            masks = pp.tile([P, P], BF16, tag="masks")
            ones = pp.tile([P, P], BF16, tag="ones")
            wqkts = [pp.tile([P, DT, P], BF16, tag=f"wqk{ct}", name=f"wqk{ct}")
                     for ct in range(NCT)]
            wvts = [pp.tile([P, DT // 2, HLOC * HD], BF16, tag=f"wv{i}",
                            name=f"wv{i}") for i in range(2)]
            wots = [pp.tile([P, D], BF16, tag=f"wo{kt}", name=f"wo{kt}")
                    for kt in range(HLOC)]
            cos_t = pp.tile([P, S], F32, tag="cos")
            sin_t = pp.tile([P, S], F32, tag="sin")

            nc.vector.memset(ones[:], 1.0)

            def o_proj(ic, aouts):
                for st in range(CHUNK // P):
                    osb = op_.tile([P, D // CHUNK, CHUNK], BF16, tag="osb")
                    for dc in range(D // CHUNK):
                        ps = psA.tile([P, CHUNK], F32, tag="psA")
                        for kt in range(HLOC):
                            nc.tensor.matmul(
                                ps[:], aouts[kt][:, P * st:P * (st + 1)],
                                wots[kt][:, CHUNK * dc:CHUNK * (dc + 1)],
                                start=(kt == 0), stop=(kt == HLOC - 1))
                        if dc % 2 == 0:
                            nc.vector.tensor_copy(osb[:, dc, :], ps[:])
                        else:
                            nc.scalar.copy(osb[:, dc, :], ps[:])
                    nc.scalar.dma_start(
                        o_r[:, ic * (CHUNK // P) + st, :], osb[:])

            def rope_store(ps_pe, dst_pe, cos_c, sin_c):
                """ps_pe: [64, CHUNK] psum AP at partition base 64 (pre-rope pe
                rows of one head). 4 DVE ops; sign-baked sin tables make the
                NeoX rotation a mult/mult/mult/add. dst_pe = rows [64:128]."""
                b = 64
                tmp = rp.tile([P, CHUNK], F32, tag="ropetmp")
                scr = rp.tile([P, CHUNK], F32, tag="ropescr")
                nc.vector.tensor_tensor(tmp[b:b + 32, :], ps_pe[32:64, :],
                                        sin_c[b:b + 32, :], mybir.AluOpType.mult)
                nc.vector.tensor_tensor(tmp[b + 32:b + 64, :], ps_pe[0:32, :],
                                        sin_c[b + 32:b + 64, :],
                                        mybir.AluOpType.mult)
                nc.vector.tensor_tensor(scr[b:b + 64, :], ps_pe[:],
                                        cos_c[b:b + 64, :], mybir.AluOpType.mult)
                nc.vector.tensor_tensor(dst_pe, scr[b:b + 64, :],
                                        tmp[b:b + 64, :], mybir.AluOpType.add)

            # ---------------- chunk loop ----------------
            for ic in range(NCHUNK):
                sl = slice(ic * CHUNK, (ic + 1) * CHUNK)

                xcs = [ap_.tile([P, DT // 2, CHUNK], BF16, tag=f"xc{i}",
                                name=f"xc{i}") for i in range(2)]
                nc.sync.dma_start(xcs[0][:], xt_r[:, ic, 0])
                nc.sync.dma_start(xcs[1][:], xt_r[:, ic, 1])

                def xsl(dt_, cols=slice(None)):
                    return xcs[dt_ // (DT // 2)][:, dt_ % (DT // 2), cols]

                if ic == 0:
                    for ct in range(NCT):
                        nc.gpsimd.dma_start(wqkts[ct][:], wqk_r[:, ct])
                    nc.scalar.dma_start(cos_t[64:128, :], cosr[:])
                    nc.scalar.dma_start(sin_t[64:128, :], sinr[:])
                    nc.scalar.dma_start(masks[:], maskd[:])
                    nc.gpsimd.dma_start(wvts[0][:], wv_r[:, 0])
                    nc.gpsimd.dma_start(wvts[1][:], wv_r[:, 1])

                cos_c = cos_t[:, sl]
                sin_c = sin_t[:, sl]

                # ---- projection: c-tile ct = head [nope64 | pe64] ----
                # ct 0..3 -> q heads, ct 4..7 -> k heads (identical rope)
                qTis = [ap_.tile([P, CHUNK], BF16, tag=f"qTi{h}", name=f"qTi{h}")
                        for h in range(HLOC)]
                for ct in range(NCT):
                    ps = psA.tile([P, CHUNK], F32, tag="psA")
                    for dt_ in range(DT):
                        nc.tensor.matmul(
                            ps[:], wqkts[ct][:, dt_, :], xsl(dt_),
                            start=(dt_ == 0), stop=(dt_ == DT - 1))
                    if ct < HLOC:
                        dst_nope = qTis[ct][0:64, :]
                        dst_pe = qTis[ct][64:128, :]
                    else:
                        dst_nope = kTs[ct - HLOC][0:64, sl]
                        dst_pe = kTs[ct - HLOC][64:128, sl]
                    nc.vector.tensor_copy(dst_nope, ps[0:64, :])
                    rope_store(ps[64:128, :], dst_pe, cos_c, sin_c)

                if ic == 0:
                    # resident Wo load, deferred so it doesn't crowd the
                    # critical first-chunk x/weight DMAs
                    for kt in range(HLOC):
                        nc.gpsimd.dma_start(wots[kt][:], wo_r[:, kt])

                # ---- v natural [CHUNK, 512]: x seq-tile stationary ----
                for st in range(CHUNK // P):
                    ps = psA.tile([P, HLOC * HD], F32, tag="psA")
                    for dt_ in range(DT):
                        nc.tensor.matmul(
                            ps[:], xsl(dt_, slice(P * st, P * (st + 1))),
                            wvts[dt_ // (DT // 2)][:, dt_ % (DT // 2), :],
                            start=(dt_ == 0), stop=(dt_ == DT - 1))
                    nc.vector.tensor_copy(vnat[:, ic * (CHUNK // P) + st, :],
                                          ps[:])

                # ---- o-projection of the PREVIOUS chunk: PE work to cover
                # the DVE rope/normalize backlog of this chunk's projections
                if ic > 0:
                    o_proj(ic - 1, prev_aouts)

                # ---- attention for this query chunk ----
                # diagonal j-tiles shrink to queries >= P*r (the rest are
                # fully masked and contribute exact zeros by omission)
                aouts = [aop.tile([P, CHUNK], BF16, tag=f"aout{h}", name=f"aout{h}")
                         for h in range(HLOC)]
                jt_max = (ic + 1) * (CHUNK // P)
                for h in range(HLOC):
                    psd = psD.tile([P, CHUNK], F32, tag="psD")
                    pso = psO.tile([P, CHUNK], F32, tag="psO")
                    for jt in range(jt_max):
                        r = jt - ic * (CHUNK // P)
                        q0 = P * r if r > 0 else 0
                        pss = psS.tile([P, CHUNK], F32, tag="psS")
                        nc.tensor.matmul(
                            pss[:, q0:], kTs[h][:, P * jt:P * (jt + 1)],
                            qTis[h][:, q0:], start=True, stop=True)
                        at = atp.tile([P, CHUNK], BF16, tag="attnT")
                        nc.scalar.activation(
                            at[:, q0:], pss[:, q0:],
                            mybir.ActivationFunctionType.Exp, scale=SCALE)
                        if r >= 0:  # triangular mask on the diagonal subtile
                            nc.vector.tensor_tensor(
                                at[:, q0:q0 + P], at[:, q0:q0 + P], masks[:],
                                mybir.AluOpType.mult)
                        nc.tensor.matmul(psd[:, q0:], ones[:], at[:, q0:],
                                         start=(jt == 0), stop=(jt == jt_max - 1))
                        nc.tensor.matmul(
                            pso[:, q0:], vnat[:, jt, HD * h:HD * (h + 1)],
                            at[:, q0:],
                            start=(jt == 0), stop=(jt == jt_max - 1))
                    rec = atp.tile([P, CHUNK], F32, tag="recip")
                    nc.vector.reciprocal_approx_fast(rec[:], psd[:])
                    nc.vector.tensor_tensor(aouts[h][:], pso[:], rec[:],
                                            mybir.AluOpType.mult)
                prev_aouts = aouts

            o_proj(NCHUNK - 1, prev_aouts)
    nc.compile()
    return nc


_NC = None


def _get_nc():
    global _NC
    if _NC is None:
        _NC = _build()
    return _NC


def _host_prep(x, Wq_down, Wq_up, Wq_rope, Wkv_down, Wk_up, Wk_rope, Wv_up, Wo):
    """Build the 8 per-core input maps (all host-side layout prep)."""
    # rope tables for SBUF partition rows 64:128 (the pe rows), NeoX sign
    # baked into sin
    half = RD // 2
    inv_freq = 1.0 / (BASE ** (np.arange(half, dtype=np.float64) / half))
    ang = np.arange(S, dtype=np.float64)[None, :] * inv_freq[:, None]  # [32, S]
    cos32 = np.cos(ang)
    sin32 = np.sin(ang)
    cosr = np.tile(cos32, (2, 1)).astype(np.float32)                   # [64,S]
    sinr = np.concatenate([-sin32, sin32], 0).astype(np.float32)

    # triangular mask for the 128x128 diagonal subtile: key p <= query i
    pidx = np.arange(P)[:, None]
    iidx = np.arange(P)[None, :]
    maskd = (pidx <= iidx).astype(_BF16)

    # fuse the low-rank compositions once, in f32
    Wfq = Wq_down @ Wq_up        # [D, H*ND]
    Wfqr = Wq_down @ Wq_rope     # [D, H*RD]
    Wfk = Wkv_down @ Wk_up       # [D, H*ND]
    Wfv = Wkv_down @ Wv_up       # [D, H*HD]

    # per-batch pre-tiled x^T: [p, ic, dt, s]
    xts = [np.ascontiguousarray(
        x[b].reshape(NCHUNK, CHUNK, DT, P).transpose(3, 0, 2, 1)
    ).reshape(P, -1).astype(_BF16) for b in range(B)]

    in_maps = []
    for c in range(8):
        b, g = divmod(c, 4)
        heads = range(HLOC * g, HLOC * (g + 1))
        Wqk = np.empty((D, NCT * P), np.float32)
        for i, h in enumerate(heads):
            q0, k0 = i * HD, HLOC * HD + i * HD
            Wqk[:, q0:q0 + ND] = Wfq[:, h * ND:(h + 1) * ND]
            Wqk[:, q0 + ND:q0 + HD] = Wfqr[:, h * RD:(h + 1) * RD]
            Wqk[:, k0:k0 + ND] = Wfk[:, h * ND:(h + 1) * ND]
            Wqk[:, k0 + ND:k0 + HD] = Wk_rope[:, h * RD:(h + 1) * RD]
        Wv = Wfv[:, g * HLOC * HD:(g + 1) * HLOC * HD]
        Wop = Wo[g * HLOC * HD:(g + 1) * HLOC * HD, :]
        in_maps.append({
            "xt": xts[b],
            "wqk": np.ascontiguousarray(
                Wqk.reshape(DT, P, NCT, P).transpose(1, 2, 0, 3)
            ).reshape(P, -1).astype(_BF16),
            "wv": np.ascontiguousarray(
                Wv.reshape(DT, P, HLOC * HD).transpose(1, 0, 2)
            ).reshape(P, -1).astype(_BF16),
            "wo": np.ascontiguousarray(
                Wop.reshape(HLOC, P, D).transpose(1, 0, 2)
            ).reshape(P, -1).astype(_BF16),
            "cosr": cosr,
            "sinr": sinr,
            "maskd": maskd,
        })
    return in_maps


def kernel(x, Wq_down, Wq_up, Wq_rope, Wkv_down, Wk_up, Wk_rope, Wv_up, Wo,
           _trace=False, _trace_kwargs=None):
    x = np.asarray(x, dtype=np.float32)
    args = [np.asarray(a, dtype=np.float32) for a in
            (Wq_down, Wq_up, Wq_rope, Wkv_down, Wk_up, Wk_rope, Wv_up, Wo)]
    in_maps = _host_prep(x, *args)
    nc = _get_nc()
    res = run_bass_kernel_spmd(nc, in_maps, core_ids=list(range(8)),
                               trace=_trace, **(_trace_kwargs or {}))
    kernel._last_results = res
    out = np.zeros((B, S, D), np.float32)
    for c in range(8):
        # un-tile [p, st, d] -> [st*128+p, d]
        part = res.results[c]["o_part"].reshape(P, S // P, D)
        out[c // 4] += part.transpose(1, 0, 2).reshape(S, D).astype(np.float32)
    return out
